# revision 62
# baseline (speedup 1.0000x reference)
"""Trainium2 Bass kernel for nn_MetricSelfAttention.

Math: the reference's softmax is dead code, so
    nudged = (p1 @ M @ p2^T) @ p1
reassociates to
    nudged = p1 @ (M @ (p2^T @ p1))        (per-head 64x64 Gram matrix G)
collapsing the O(W^2) attention matrices entirely, and the mixer folds to
    out = p1u @ Whm,   Whm = H_bd @ Wmix_slice   (precomputed once on-chip)
so the back end is a single matmul per w-tile straight from p1^T.

Sharding: 8 cores = 2 batches x 4 head-pairs.  Core (b, hg) computes heads
{2hg, 2hg+1} of batch b and the partial output; the host sums the 4 partials
per batch and adds b_mixer.  All activations ship in partition-contiguous
layouts (pure layout/dtype prep on host) so every DMA is 4KB-contiguous per
partition; the partial outputs are written partition-major and unscrambled
on the host.

LayerNorm folding (no normalized tensors are materialized):
  - gamma folds into the projection on the host; nonzero beta enters as
    rank-1 bias matmuls; omitted entirely when beta == 0.
  - x1 row stats come from x1^T via PE ones-matmuls producing rows at
    partitions 0/32 of one PSUM bank, which are PE-transposed to columns
    ([33,128] transposes) so all variance/sqrt/reciprocal arithmetic runs
    128-lane parallel.
  - rstd1 never exists in row form: p1^T stays UNSCALED; rstd1*rstd2 scales
    the transposed p1s columns (0-stride-broadcast paired multiply), and
    rstd1 alone scales the output-stage PSUM evacuation (per-partition).
    Mean-subtraction enters the projection as one rank-1 matmul
    (-colsum) (x) mu_row with colsum negated on the host.
  - x2 is never normalized or transposed; its mean-removal is a rank-1
    correction applied to G (not F):
        G -= colsumP (x) v,   v[k] = sum_w mu2[w] p1s[w,k]
    with v accumulated from one F=128 matmul per w-tile (lhsT = mu2
    column) -- far cheaper than the 16 F=512 broadcast matmuls the
    F-level correction needs.

A short burst of dummy matmuls at t=0 trips the PE HAM clock gate to 8/8
before real work arrives (otherwise the first ~3.4us of matmuls run at
1.2 GHz).

All tensors flow as bf16 (PSUM accumulation in fp32); partial outputs are
written bf16 and summed in fp32 on the host.
"""

from contextlib import ExitStack

import numpy as np
import ml_dtypes

import concourse.bacc as bacc
import concourse.bass as bass
import concourse.tile as tile
from concourse import mybir
from concourse.bass_utils import run_bass_kernel_spmd
from concourse.masks import make_identity

B, W, C, N, K = 2, 2048, 512, 8, 64
NCORES = 8
HPC = 2          # heads per core
K2 = HPC * K     # 128 channels per core
EPS = 1e-5
FP32 = mybir.dt.float32
BF16 = mybir.dt.bfloat16
NPBF16 = ml_dtypes.bfloat16
AF = mybir.ActivationFunctionType

NT = W // 128    # 16 w-tiles
NQ = W // 512    # 4 w-quads
NJ = C // 128    # 4 c-chunks

N_WARM = 0       # PE warm-up matmuls (~3.4us at 1.2GHz trips the HAM gate)


def _free_bcast(ap2d, n):
    """[128, m] AP -> [128, m, n] view with 0-stride innermost dim."""
    return bass.AP(tensor=ap2d.tensor, offset=ap2d.offset,
                   ap=list(ap2d.ap) + [[0, n]])


def _body(ctx: ExitStack, tc: tile.TileContext, x1td, x2d, projd,
          wmixd, pcolsumd, outd, pbiasrd):
    nc = tc.nc
    with_pbias = pbiasrd is not None

    persist = ctx.enter_context(tc.tile_pool(name="persist", bufs=1))
    sqpool = ctx.enter_context(tc.tile_pool(name="sq", bufs=3))
    rowpool = ctx.enter_context(tc.tile_pool(name="rows", bufs=3))
    spool = ctx.enter_context(tc.tile_pool(name="stats", bufs=4))
    outpool = ctx.enter_context(tc.tile_pool(name="outstage", bufs=4))
    ps_tp = ctx.enter_context(tc.tile_pool(
        name="ps_tp", bufs=1 if with_pbias else 2, space="PSUM"))
    ps_acc = ctx.enter_context(tc.tile_pool(name="ps_acc", bufs=1, space="PSUM"))
    ps_st = ctx.enter_context(tc.tile_pool(name="ps_st", bufs=1, space="PSUM"))
    ps_mm = ctx.enter_context(tc.tile_pool(name="ps_mm", bufs=2, space="PSUM"))
    ps_mo = ctx.enter_context(tc.tile_pool(name="ps_mo", bufs=2, space="PSUM"))

    # ---- constants (no DMA dependence; available immediately) -------------
    # make_identity runs FIRST on gpsimd so warm-up matmuls and transposes
    # never queue behind DMA-issue instructions.
    # Warm-up operand: a zero tile needing only ONE memset (no
    # make_identity dependency), so warm-up matmuls fire the moment the
    # engines come up.
    warm_w = persist.tile([128, 128], BF16)
    nc.vector.memset(warm_w, 0.0)
    warm4 = bass.AP(tensor=warm_w.tensor, offset=warm_w.offset,
                    ap=[warm_w.ap[0], [0, 4], warm_w.ap[1]])  # [128,4,128]

    def warm_mm(n, tag="mo"):
        pool = ps_mo if tag == "mo" else ps_tp
        for _ in range(n):
            if tag == "mo":
                wps = pool.tile([128, 512], FP32, tag=tag)
                nc.tensor.matmul(wps, lhsT=warm_w, rhs=warm4,
                                 start=True, stop=True)
            else:
                wps = pool.tile([128, 2, 128], FP32, tag=tag)
                nc.tensor.matmul(wps[:, 0, :], lhsT=warm_w, rhs=warm_w,
                                 start=True, stop=True)

    # ---- PE warm-up: trips the HAM clock gate during the DMA-in phase -----
    warm_mm(N_WARM)

    eps_s = persist.tile([128, 1], FP32)
    nc.vector.memset(eps_s, EPS)
    oneshalf = persist.tile([128, 1], BF16)      # +1/C (ssq row)
    nc.vector.memset(oneshalf, 1.0 / C)
    negoneshalf = persist.tile([128, 1], BF16)   # -1/C (mu row, negated)
    nc.vector.memset(negoneshalf, -1.0 / C)
    ident = persist.tile([128, 128], BF16)
    nc.vector.memset(ident, 0.0)
    make_identity(nc, ident, nomemset=True)

    # ---- persistent activations (loads issued immediately) ----------------
    # One SWDGE queue sustains only ~60 GB/s, so the loads are split into
    # per-chunk DMAs spread over 4 engine queues: x1t chunk j of each quad
    # goes on engine j's queue (sync/scalar/vector/gpsimd), x2 quads split
    # gpsimd/vector.  Quad-0 inputs land ~3us in instead of ~16us.
    proj_s = persist.tile([128, NJ, K2], BF16)
    nc.sync.dma_start(out=proj_s, in_=projd)
    x1t_r = persist.tile([128, NQ, NJ, 512], BF16)
    x2r_s = persist.tile([128, NT, C], BF16)

    def issue_x1t(q):
        nc.sync.dma_start(out=x1t_r[:, q, 0, :], in_=x1td[:, q, 0, :])
        nc.scalar.dma_start(out=x1t_r[:, q, 1, :], in_=x1td[:, q, 1, :])
        nc.scalar.dma_start(out=x1t_r[:, q, 3, :], in_=x1td[:, q, 3, :])
        nc.gpsimd.dma_start(out=x1t_r[:, q, 2, :], in_=x1td[:, q, 2, :])

    # x2 ships as half-quad pieces spread over all three queues (always
    # BEHIND x1t in each queue's order) so no single queue carries 2.5MB
    # and late x1t chunks stop starving the PE front end.
    x2eng = {(0, 0): nc.sync, (0, 1): nc.gpsimd,
             (1, 0): nc.scalar, (1, 1): nc.sync,
             (2, 0): nc.gpsimd, (2, 1): nc.scalar,
             (3, 0): nc.sync, (3, 1): nc.gpsimd}

    def issue_x2(q):
        for h in range(2):
            t0 = 4 * q + 2 * h
            x2eng[(q, h)].dma_start(out=x2r_s[:, t0:t0 + 2, :],
                                    in_=x2d[:, t0:t0 + 2, :])

    # quads 0-1 up front; later quads staggered from inside the loop so the
    # ~0.7us-per-issue cost doesn't serialize ahead of early compute.
    # All x1t before any x2 on the shared queues: bn-stats results are not
    # consumed until 1-2 iterations later, x1t chunks gate the PE directly.
    issue_x1t(0)
    issue_x1t(1)
    pcolsum_s = persist.tile([1, K2], BF16)     # positive colsum (host)
    nc.sync.dma_start(out=pcolsum_s, in_=pcolsumd)
    issue_x2(0)
    issue_x2(1)

    wmm_s = persist.tile([128, C + HPC * K], BF16)
    wmix_s = wmm_s[:, 0:C]
    pbiasr_s = None
    if with_pbias:
        pbiasr_s = persist.tile([1, K2], BF16)
        nc.scalar.dma_start(out=pbiasr_s, in_=pbiasrd)

    ht_bd_s = persist.tile([K2, K2], BF16)      # block-diag H^T = (M G)^T
    nc.vector.memset(ht_bd_s, 0.0)

    # ---- persistent activations --------------------------------------------
    rstd2_s = persist.tile([128, NT], FP32)     # per-row 1/std of x2
    rstd1_s = persist.tile([128, NT], FP32)     # per-row 1/std of x1
    sc12_s = persist.tile([128, NT], FP32)      # rstd1*rstd2 per row
    p1s_s = persist.tile([128, NT, K2], BF16)   # rstd1*rstd2 * p1u (rowwise)
    p1T_s = persist.tile([K2, W], BF16)         # p1^T UNSCALED (p1u^T)
    ft_s = persist.tile([K2, C], BF16)          # F^T (x2 mean NOT removed)
    f_s = persist.tile([128, NJ, K2], BF16)     # F (c on partitions)
    g_s = persist.tile([K, HPC, K], BF16)       # per-head Gram
    whm_s = persist.tile([K2, C], BF16)         # Whm = H_bd @ Wmix_slice
    if with_pbias:
        std2_s = persist.tile([128, NT], BF16)  # per-row std of x2
        s1_s = persist.tile([1, K2], BF16)      # column sums of p1

    def p1s_pairs(q):
        # paired PE-transposes + 0-stride-broadcast scale for quad q
        for tp in range(2):
            w0 = 4 * q + 2 * tp
            ps2 = ps_tp.tile([128, 2, 128], BF16, tag="tp")
            for i in range(2):
                nc.tensor.transpose(
                    ps2[:, i, :],
                    p1T_s[:, (w0 + i) * 128:(w0 + i + 1) * 128], ident)
            nc.vector.tensor_mul(p1s_s[:, w0:w0 + 2, :], ps2,
                                 _free_bcast(sc12_s[:, w0:w0 + 2], K2))

    whm_s = persist.tile([K2, C], BF16)         # Whm = H_bd @ Wmix_slice
    whm_s = persist.tile([K2, C], BF16)         # Whm = H_bd @ Wmix_slice
    facc_a = ps_acc.tile([128, 512], FP32, tag="facc")

    def wrow_f_partials(q):
        # F^T += p1s_t^T @ x2_t  (raw x2; mean handled as rank-1 on G)
        for t in range(4 * q, 4 * q + 4):
            nc.tensor.matmul(facc_a, lhsT=p1s_s[:, t, :], rhs=x2r_s[:, t, :],
                             start=(t == 0), stop=(t == NT - 1))

    # ========================================================================
    # Front end: 4-deep software pipeline over w-quads.  Stage lag keeps the
    # in-order PE stream from ever blocking on a cross-engine chain:
    #   iter i:  squares/bn/proj/stats(i) | strow+colsum+stat-tp+chain(i-1)
    #            | p1s+v(i-2) | F(i-3)
    # ========================================================================
    pts = [None] * NQ

    sq_list = [None] * NQ

    def s_sq(q):
        # x1 squares (feed ssq matmuls), computed one iteration AHEAD of
        # the stats matmuls that consume them so the PE never waits.
        sqt = sqpool.tile([128, NJ, 512], BF16, tag="sq")
        sq_list[q] = sqt
        for j in range(2):
            nc.scalar.activation(sqt[:, j, :], x1t_r[:, q, j, :], AF.Square)
        nc.vector.tensor_mul(sqt[:, 2, :], x1t_r[:, q, 2, :],
                             x1t_r[:, q, 2, :])
        nc.gpsimd.tensor_mul(sqt[:, 3, :], x1t_r[:, q, 3, :],
                             x1t_r[:, q, 3, :])

    def s0_front(q):
        q4 = slice(4 * q, 4 * (q + 1))
        sqt = sq_list[q]

        # x2 row stats: bn per tile, tail ops batched over the quad
        mv = spool.tile([128, 4, 2], FP32, tag="mv")
        for t in range(4):
            stats = spool.tile([128, 6], FP32, tag="bst")
            nc.vector.bn_stats(stats, x2r_s[:, 4 * q + t, :])
            nc.vector.bn_aggr(mv[:, t, :], stats)
        std2q = spool.tile([128, 4], FP32, tag="stdq")
        nc.scalar.activation(std2q, mv[:, :, 1], AF.Sqrt, bias=eps_s, scale=1.0)
        nc.vector.reciprocal(rstd2_s[:, q4], std2q)
        if with_pbias:
            nc.gpsimd.tensor_copy(out=std2_s[:, q4], in_=std2q)

        # PE: projection mains + x1 stat rows (partitions 0/32)
        pt = ps_mm.tile([128, 512], FP32, tag="mm")
        pts[q] = pt
        for j in range(NJ):
            nc.tensor.matmul(pt, lhsT=proj_s[:, j, :], rhs=x1t_r[:, q, j, :],
                             start=(j == 0), stop=False)
        st_ps = ps_mo.tile([128, 512], FP32, tag="mo")
        mu_psv = st_ps[0:1, :]       # NEGATED mu row (lhsT = -1/C)
        ssq_psv = st_ps[32:33, :]
        for j in range(NJ):
            nc.tensor.matmul(mu_psv, lhsT=negoneshalf, rhs=x1t_r[:, q, j, :],
                             start=(j == 0), stop=(j == NJ - 1))
            nc.tensor.matmul(ssq_psv, lhsT=oneshalf, rhs=sqt[:, j, :],
                             start=(j == 0), stop=(j == NJ - 1))
        return st_ps

    strow_list = [None] * NQ

    def s_strow(q, st_ps):
        # emitted at the HEAD of the next iteration's ACT stream so it is
        # done well before the colsum/stat-transpose matmuls need it
        strow = rowpool.tile([33, 512], BF16, tag="strow")
        strow_list[q] = strow
        nc.scalar.copy(out=strow, in_=st_ps[0:33, :])

    def s1_stats(q, st_ps):
        q4 = slice(4 * q, 4 * (q + 1))
        pt = pts[q]
        strow = strow_list[q]

        # close projection: += colsum (x) (-mu_row)  (mu row is negated)
        nc.tensor.matmul(pt, lhsT=pcolsum_s, rhs=strow[0:1, :],
                         start=False, stop=not with_pbias)

        # merged stat transposes: [33,128] -> [128,33] columns
        stc = ps_st.tile([128, 4, 34], BF16, tag="st")
        for t in range(4):
            nc.tensor.transpose(stc[:, t, 0:33],
                                strow[:, t * 128:(t + 1) * 128],
                                ident[0:33, 0:33])
        stq = spool.tile([128, 4, 34], BF16, tag="stq")
        nc.vector.tensor_copy(out=stq, in_=stc)
        musq = spool.tile([128, 4], FP32, tag="musq")
        nc.vector.tensor_mul(musq, stq[:, :, 0], stq[:, :, 0])
        varq = spool.tile([128, 4], FP32, tag="varq")
        nc.vector.tensor_sub(varq, stq[:, :, 32], musq)
        std1q = spool.tile([128, 4], FP32, tag="std1")
        nc.scalar.activation(std1q, varq, AF.Sqrt, bias=eps_s, scale=1.0)
        nc.vector.reciprocal(rstd1_s[:, q4], std1q)
        nc.vector.tensor_mul(sc12_s[:, q4], rstd1_s[:, q4], rstd2_s[:, q4])

        if with_pbias:
            # p1 = rstd1*p1u + 1 (x) pbias => fold as p1u += std1 (x) pbias
            std1q_b = spool.tile([128, 4], BF16, tag="s1qb")
            nc.gpsimd.tensor_copy(out=std1q_b, in_=std1q)
            s1r_ps = ps_tp.tile([1, 512], BF16, tag="s1rp")
            for t in range(4):
                nc.tensor.transpose(s1r_ps[:, t * 128:(t + 1) * 128],
                                    std1q_b[:, t:t + 1], ident)
            std1row = rowpool.tile([1, 512], BF16, tag="s1r")
            nc.gpsimd.tensor_copy(out=std1row, in_=s1r_ps)
            nc.tensor.matmul(pt, lhsT=pbiasr_s, rhs=std1row,
                             start=False, stop=True)

        nc.scalar.copy(out=p1T_s[:, q * 512:(q + 1) * 512], in_=pt)

    st_list = [None] * NQ
    s_sq(0)
    for q in range(NQ):
        if q > 0:
            s_strow(q - 1, st_list[q - 1])
        if q + 1 < NQ:
            s_sq(q + 1)
        if q + 2 < NQ:
            issue_x1t(q + 2)
            issue_x2(q + 2)
        if q == 1:
            nc.sync.dma_start(out=wmm_s, in_=wmixd)
        st_list[q] = s0_front(q)
        if q > 0:
            s1_stats(q - 1, st_list[q - 1])
        if q > 1:
            p1s_pairs(q - 2)
        if q > 2:
            wrow_f_partials(q - 3)
    s_strow(NQ - 1, st_list[NQ - 1])
    s1_stats(NQ - 1, st_list[NQ - 1])
    p1s_pairs(NQ - 2)
    wrow_f_partials(NQ - 3)
    p1s_pairs(NQ - 1)
    wrow_f_partials(NQ - 2)

    # ========================================================================
    # Gram tail
    # ========================================================================
    wrow_f_partials(NQ - 1)

    gp = ps_mm.tile([128, 512], FP32, tag="mm")
    gpv = gp[:, :K2]

    def f_to_gram(ft_tile, f_tile, facc, first):
        nc.vector.tensor_copy(out=ft_tile[:, 0:256], in_=facc[:, 0:256])
        nc.scalar.copy(out=ft_tile[:, 256:512], in_=facc[:, 256:512])
        for jp in range(2):
            fjp = ps_tp.tile([128, 2, 128], BF16, tag="tp")
            for i in range(2):
                j = 2 * jp + i
                nc.tensor.transpose(fjp[:, i, :],
                                    ft_tile[:, j * 128:(j + 1) * 128], ident)
            if jp == 0:
                nc.vector.tensor_copy(out=f_tile[:, 0:2, :], in_=fjp)
            else:
                nc.scalar.copy(out=f_tile[:, 2:4, :], in_=fjp)
        for j in range(NJ):
            nc.tensor.matmul(gpv, lhsT=proj_s[:, j, :], rhs=f_tile[:, j, :],
                             start=(first and j == 0), stop=False)

    f_to_gram(ft_s, f_s, facc_a, True)

    # s1 = column sums of p1 = std2^T @ p1s (beta rank-1 term in G)
    if with_pbias:
        sp = ps_mo.tile([128, 512], FP32, tag="mo")
        spv = sp[:1, :K2]
        for t in range(NT):
            nc.tensor.matmul(spv, lhsT=std2_s[:, t:t + 1], rhs=p1s_s[:, t, :],
                             start=(t == 0), stop=(t == NT - 1))
        nc.vector.tensor_copy(out=s1_s, in_=spv)

    # x2 mean-removal enters as one rank-1 matmul: G += pcolsum (x) (-v).
    # v falls out of F_raw for free:  sum_c F_raw^T[k,c] = C * v[k],
    # so one free-dim reduce over the evacuated ft replaces the 16
    # per-tile v matmuls (and the mu2 machinery) entirely.
    vsum = spool.tile([128, 1], FP32, tag="vsum")
    nc.vector.tensor_reduce(vsum, ft_s, axis=mybir.AxisListType.X,
                            op=mybir.AluOpType.add)
    vsb = rowpool.tile([128, 1], BF16, tag="vsb")
    nc.vector.tensor_scalar_mul(vsb, vsum, -1.0 / C)
    vrow_ps = ps_tp.tile([1, K2], FP32, tag="tp")
    nc.tensor.matmul(vrow_ps, lhsT=vsb, rhs=ident, start=True, stop=True)
    vneg_b = rowpool.tile([1, K2], BF16, tag="vnb")
    nc.vector.tensor_copy(out=vneg_b, in_=vrow_ps)
    nc.tensor.matmul(gpv, lhsT=pcolsum_s, rhs=vneg_b,
                     start=False, stop=not with_pbias)
    if with_pbias:
        # += pbias (x) s1 ; diagonal blocks get pbias_h (x) s1_h
        nc.tensor.matmul(gpv, lhsT=pbiasr_s, rhs=s1_s, start=False, stop=True)
    for h in range(HPC):
        nc.vector.tensor_copy(out=g_s[:, h, :],
                              in_=gpv[h * K:(h + 1) * K, h * K:(h + 1) * K])

    # H^T_h = G_h^T @ M_h (M symmetric); assemble block-diag H^T
    # (M_h lives in wmm_s[0:K, C+h*K : C+(h+1)*K])
    hp = ps_mm.tile([128, 512], FP32, tag="mm")
    for h in range(HPC):
        nc.tensor.matmul(hp[h * K:(h + 1) * K, :K], lhsT=g_s[:, h, :],
                         rhs=wmm_s[0:K, C + h * K:C + (h + 1) * K])
    for h in range(HPC):
        nc.vector.tensor_copy(out=ht_bd_s[h * K:(h + 1) * K, h * K:(h + 1) * K],
                              in_=hp[h * K:(h + 1) * K, :K])

    # Whm = H_bd @ Wmix_slice  (lhsT = H^T_bd)
    whp = ps_mm.tile([128, 512], FP32, tag="mm")
    nc.tensor.matmul(whp, lhsT=ht_bd_s, rhs=wmix_s)
    nc.vector.tensor_copy(out=whm_s[:, 0:256], in_=whp[:, 0:256])
    nc.scalar.copy(out=whm_s[:, 256:512], in_=whp[:, 256:512])

    # out = rstd1 * (p1u @ Whm), evacuated with per-partition rstd1 scale.
    # Output is written partition-major ([128, NT, C]); host unscrambles.
    # Staged and stored per tile-PAIR so stores start as early as possible,
    # alternating the two store queues.
    for p in range(2 * NQ):
        stage = outpool.tile([128, 2, C], BF16, tag="ostage")
        for i in range(2):
            w_t = 2 * p + i
            pool = ps_mo if w_t % 2 == 0 else ps_mm
            mo = pool.tile([128, 512], FP32,
                           tag="mo" if w_t % 2 == 0 else "mm")
            nc.tensor.matmul(mo, lhsT=p1T_s[:, w_t * 128:(w_t + 1) * 128],
                             rhs=whm_s)
            if (w_t % 4) in (0, 2) and not (w_t % 8 == 0):
                nc.vector.tensor_scalar_mul(stage[:, i, :], mo,
                                            rstd1_s[:, w_t:w_t + 1])
            else:
                nc.scalar.activation(stage[:, i, :], mo, AF.Copy,
                                     scale=rstd1_s[:, w_t:w_t + 1])
        eng = nc.sync if p % 2 == 0 else nc.scalar
        eng.dma_start(out=outd[:, 2 * p:2 * p + 2, :], in_=stage)


_PROGRAM_CACHE = {}


def _get_program(with_pbias: bool):
    key = ("nc", with_pbias)
    if key in _PROGRAM_CACHE:
        return _PROGRAM_CACHE[key]
    nc = bacc.Bacc("TRN2", debug=False, num_devices=NCORES)
    x1td = nc.dram_tensor("x1t", [128, NQ, NJ, 512], BF16,
                          kind="ExternalInput").ap()
    x2d = nc.dram_tensor("x2", [128, NT, C], BF16, kind="ExternalInput").ap()
    projd = nc.dram_tensor("proj", [128, NJ, K2], BF16,
                           kind="ExternalInput").ap()
    wmixd = nc.dram_tensor("wmm", [128, C + HPC * K], BF16,
                           kind="ExternalInput").ap()
    pcolsumd = nc.dram_tensor("pcolsum", [1, K2], BF16,
                              kind="ExternalInput").ap()
    pbiasrd = None
    if with_pbias:
        pbiasrd = nc.dram_tensor("pbiasr", [1, K2], BF16, kind="ExternalInput").ap()
    outd = nc.dram_tensor("out", [128, NT, C], BF16, kind="ExternalOutput").ap()
    with tile.TileContext(nc) as tc:
        with ExitStack() as ctx:
            _body(ctx, tc, x1td, x2d, projd, wmixd, pcolsumd,
                  outd, pbiasrd)
    nc.compile()
    _PROGRAM_CACHE[key] = nc
    return nc


def _host_prep(inputs):
    x1 = np.asarray(inputs["x1"], np.float32)
    x2 = np.ascontiguousarray(np.asarray(inputs["x2"], np.float32))
    gamma = np.asarray(inputs["gamma"], np.float32)
    beta = np.asarray(inputs["beta"], np.float32)
    proj = np.asarray(inputs["proj_nck"], np.float32)
    halves = np.asarray(inputs["halves"], np.float32)
    diagonals = np.asarray(inputs["diagonals"], np.float32)
    wmix = np.asarray(inputs["W_mixer"], np.float32)

    iu0, iu1 = np.triu_indices(K, k=1)
    m = np.zeros((N, K, K), np.float32)
    m[:, iu0, iu1] = halves
    m = m + np.swapaxes(m, -1, -2)
    d = np.arange(K)
    m[:, d, d] = diagonals

    pgam = proj * gamma[None, :, None]          # gamma folded into projection
    with_pbias = bool(np.any(beta))
    pbias = np.einsum("c,nck->nk", beta, proj) if with_pbias else None

    # x1^T in [128, NQ, NJ, 512]: [p,q,j,w'] = x1[b][q*512+w', j*128+p]
    x1t = [np.ascontiguousarray(
        x1[b].reshape(NQ, 512, NJ, 128).transpose(3, 0, 2, 1)).astype(NPBF16)
        for b in range(B)]
    # x2 in [128, NT, C]: [p,t,c] = x2[b][t*128+p, c]
    x2b = [np.ascontiguousarray(
        x2[b].reshape(NT, 128, C).transpose(1, 0, 2)).astype(NPBF16)
        for b in range(B)]

    in_maps = []
    for core in range(NCORES):
        b, hg = divmod(core, NCORES // B)
        h0 = HPC * hg
        proj_core = np.ascontiguousarray(
            np.concatenate([pgam[h0 + i] for i in range(HPC)], axis=1))
        proj_bf = proj_core.astype(NPBF16)
        colsum = proj_bf.astype(np.float32).sum(axis=0)
        # combined params: [128, C] wmix slice (transposed) | [128, HPC*K]
        # metric (rows 0:64, cols h*K:(h+1)*K hold head h's M; rest zero)
        mpack = np.zeros((128, HPC * K), np.float32)
        for i in range(HPC):
            mpack[0:K, i * K:(i + 1) * K] = m[h0 + i]
        wmm = np.concatenate([wmix[:, K2 * hg:K2 * (hg + 1)].T, mpack], axis=1)
        im = {
            "x1t": x1t[b],
            "x2": x2b[b],
            "proj": np.ascontiguousarray(
                proj_bf.reshape(NJ, 128, K2).transpose(1, 0, 2)),
            "wmm": np.ascontiguousarray(wmm).astype(NPBF16),
            "pcolsum": np.ascontiguousarray(colsum[None, :]).astype(NPBF16),
        }
        if with_pbias:
            pb = np.concatenate([pbias[h0 + i] for i in range(HPC)])
            im["pbiasr"] = np.ascontiguousarray(pb[None, :]).astype(NPBF16)
        in_maps.append(im)
    return in_maps, with_pbias


def kernel(**inputs) -> np.ndarray:
    in_maps, with_pbias = _host_prep(inputs)
    nc = _get_program(with_pbias)
    res = run_bass_kernel_spmd(nc, in_maps, core_ids=list(range(NCORES)))
    out = np.zeros((B, W, C), np.float32)
    for core in range(NCORES):
        b = core // (NCORES // B)
        o = res.results[core]["out"].astype(np.float32)   # [128, NT, C]
        out[b] += o.transpose(1, 0, 2).reshape(W, C)
    out += np.asarray(inputs["b_mixer"], np.float32)[None, None, :]
    return out
